# revision 1
# baseline (speedup 1.0000x reference)
"""KAN feed-forward on Trainium2 — Bass/Tile kernel, 8-core data-parallel.

Math transform: each KAN layer is
    y = silu(x) @ scale_base + einsum('nig,iog,io->no', B(x), coef, scale_sp)
with B the (G=5, K=3) uniform-grid B-spline basis (8 funcs/dim, knots
t_j = -2.2 + 0.4 j, j=0..11).  All 8 basis functions are integer shifts of the
cardinal cubic B-spline b3, and b3(t) = (1/6) sum_k (-1)^k C(4,k) relu(t-k)^3.
With u = 2.5 x + 5.5 clamped to [0, 11] (all basis functions vanish exactly at
both clamp points, so clamping is exact), the spline path becomes a dense
matmul over NM=11 truncated-power features per input dim:
    y_sp[n,o] = sum_{i,m} relu(u_ni - m)^3 * W[m,i,o],   m = 0..10
(the m=11 feature is identically zero on the clamped domain). W folds the
binomial stencil, scale_sp and coef on the host.  The silu base path rides the
same PSUM accumulation as extra K-tiles.

Per-core layout (512 tokens/core):
  L1: out1[o, tok] (+=) over 48 K-tiles (4 silu + 44 spline), lhsT = W1 tiles,
      rhs = feature tiles [128, 512] built from xT by ACT/DVE.
  L2: out2[tok, o] (natural) over 96 K-tiles, lhsT = feature tile slices,
      rhs = W2 tiles [128, 512].  L1's PSUM output [h, tok] is exactly the
      transposed layout L2's feature construction needs — no transposes.
"""

import math
import os
import sys
from contextlib import ExitStack

import numpy as np

for _p in ("/opt/trn_rl_repo",):
    if _p not in sys.path:
        sys.path.insert(0, _p)

# ---------------------------------------------------------------- constants
NG = 8  # G + K spline coefficients per edge
NM = 11  # truncated powers m = 0..10
D, H, O = 512, 1024, 512
NCORES = 8
NTOK = 4096
TOK = NTOK // NCORES  # 512 tokens per core
P = 128
UMAX = 11.0

L1_NK = 4 + NM * 4  # 48 K-tiles of 128 (4 base + 44 spline)
L2_NK = 8 + NM * 8  # 96 K-tiles of 128 (8 base + 88 spline)

# dtype toggles for matmul operands: "f32" | "f16" | "bf16".
# fp32 streams through the PE at 1/2-1/4 rate; fp16 keeps full rate with a
# 2^-12 mantissa (values here: |R| <= 1331, |W| ~ 0.5 -- well inside range).
W_MODE = os.environ.get("KAN_W_DT", "f32")
R_MODE = os.environ.get("KAN_R_DT", "f32")

# L2 sq-pass engine split: m values whose (u-m)^2 runs on ACT (Square), rest DVE
L2_SQ_ACT_M = {1, 2}

_BUILD_CACHE: dict = {}


def _np_wdt():
    if W_MODE == "bf16":
        import ml_dtypes

        return ml_dtypes.bfloat16
    if W_MODE == "f16":
        return np.float16
    return np.float32


# ---------------------------------------------------------------- host prep
def _stencil() -> np.ndarray:
    S = np.zeros((NM, NG), np.float64)
    for m in range(NM):
        k = m - np.arange(NG)
        for g in range(NG):
            kk = m - g
            if 0 <= kk <= 4:
                S[m, g] = ((-1.0) ** kk) * math.comb(4, kk) / 6.0
    return S


def _pack_w1(coef1, scale_sp1, scale_base1) -> np.ndarray:
    """-> (48, 128, 1024): k-tile, rows(K-slice), cols (ob*128+c) of hidden."""
    S = _stencil()
    A = coef1.astype(np.float64) * scale_sp1.astype(np.float64)[:, :, None]
    W1s = np.einsum("mg,iog->mio", S, A)  # (11, 512, 1024)
    w1 = np.empty((L1_NK, P, H), np.float32)
    for ib in range(4):
        w1[ib] = scale_base1[ib * P : (ib + 1) * P]
    for m in range(NM):
        for ib in range(4):
            w1[4 + m * 4 + ib] = W1s[m, ib * P : (ib + 1) * P]
    return np.ascontiguousarray(w1.astype(_np_wdt()))


def _pack_w2(coef2, scale_sp2, scale_base2) -> np.ndarray:
    """-> (96, 128, 512): K-tile rows x output cols."""
    S = _stencil()
    A = coef2.astype(np.float64) * scale_sp2.astype(np.float64)[:, :, None]
    W2s = np.einsum("mg,iog->mio", S, A)  # (11, 1024, 512)
    w2 = np.empty((L2_NK, P, O), np.float32)
    for j in range(8):
        w2[j] = scale_base2[j * P : (j + 1) * P]
    for m in range(NM):
        for j in range(8):
            w2[8 + m * 8 + j] = W2s[m, j * P : (j + 1) * P]
    return np.ascontiguousarray(w2.astype(_np_wdt()))


# ---------------------------------------------------------------- bass build
def _emit_features(nc, tmp, rp, u_tile, m, r_dt, sq_on_act, name, bias_ap):
    """Emit ops computing r = relu(u - m)^3 as a [P, free] tile; returns AP."""
    import concourse.mybir as mybir

    AF = mybir.ActivationFunctionType
    free = u_tile.shape[-1]
    if m == 0:
        s_ap = u_tile  # u >= 0 already
    else:
        s = tmp.tile([P, free], mybir.dt.float32, tag="s", name=f"s{name}m{m}")
        nc.scalar.activation(s, u_tile, AF.Relu, bias=bias_ap(float(-m)))
        s_ap = s
    sq = tmp.tile([P, free], mybir.dt.float32, tag="q", name=f"q{name}m{m}")
    if sq_on_act:
        nc.scalar.activation(sq, u_tile, AF.Square, bias=bias_ap(float(-m)))
    else:
        nc.vector.tensor_mul(sq, s_ap, s_ap)
    r = rp.tile([P, free], r_dt, tag="r", name=f"r{name}m{m}")
    nc.vector.tensor_mul(r, sq, s_ap)
    return r


def _build_kernel():
    """Build + compile the Bass program once; cached per process."""
    if "nc" in _BUILD_CACHE:
        return _BUILD_CACHE["nc"]

    import concourse.mybir as mybir
    import concourse.tile as tile
    from concourse import bacc

    AF = mybir.ActivationFunctionType
    F32 = mybir.dt.float32
    _dt = {"f32": F32, "f16": mybir.dt.float16, "bf16": mybir.dt.bfloat16}
    WDT = _dt[W_MODE]
    RDT = _dt[R_MODE]

    nc = bacc.Bacc("TRN2", target_bir_lowering=False, debug=False, num_devices=NCORES)

    xT = nc.dram_tensor("xT", (D, TOK), F32, kind="ExternalInput").ap()
    w1 = nc.dram_tensor("w1", (L1_NK, P, H), WDT, kind="ExternalInput").ap()
    w2 = nc.dram_tensor("w2", (L2_NK, P, O), WDT, kind="ExternalInput").ap()
    out = nc.dram_tensor("out", (TOK, O), F32, kind="ExternalOutput").ap()

    with tile.TileContext(nc) as tc, ExitStack() as ctx:
        persist = ctx.enter_context(tc.tile_pool(name="persist", bufs=1))
        tmp = ctx.enter_context(tc.tile_pool(name="tmp", bufs=3))
        rp = ctx.enter_context(tc.tile_pool(name="rp", bufs=4))
        w1p = ctx.enter_context(tc.tile_pool(name="w1p", bufs=4))
        w2p = ctx.enter_context(tc.tile_pool(name="w2p", bufs=6))
        outp = ctx.enter_context(tc.tile_pool(name="outp", bufs=4))
        psum = ctx.enter_context(tc.tile_pool(name="psum", bufs=1, space="PSUM"))

        _bias_cache: dict = {}

        def bias_ap(val: float):
            if val not in _bias_cache:
                t = persist.tile([P, 1], F32, tag=f"bias{len(_bias_cache)}",
                                 name=f"bias_{len(_bias_cache)}")
                nc.vector.memset(t, val)
                _bias_cache[val] = t
            return _bias_cache[val]

        # ---- L1 inputs: xT tiles + activations --------------------------
        xt = []
        for ib in range(4):
            t = persist.tile([P, TOK], F32, tag="xt", bufs=2, name=f"xt{ib}")
            nc.sync.dma_start(out=t, in_=xT[ib * P : (ib + 1) * P, :])
            xt.append(t)

        u1, si1 = [], []
        for ib in range(4):
            t1 = tmp.tile([P, TOK], F32, tag="t1", name=f"t1_{ib}")
            nc.scalar.activation(t1, xt[ib], AF.Relu, bias=bias_ap(5.5), scale=2.5)
            u = persist.tile([P, TOK], F32, tag=f"u1{ib}", name=f"u1_{ib}")
            nc.vector.tensor_scalar_min(u, t1, UMAX)
            u1.append(u)
            s = persist.tile([P, TOK], RDT, tag=f"si1{ib}", name=f"si1_{ib}")
            nc.scalar.activation(s, xt[ib], AF.Silu, bias=bias_ap(0.0))
            si1.append(s)

        # ---- L1 matmuls: out1[o_blk, tok] accumulated over 48 K-tiles ---
        pb = [
            psum.tile([P, TOK], F32, tag=f"p{ob}", name=f"p{ob}") for ob in range(8)
        ]

        def l1_block(k, rhs_ap):
            wt = w1p.tile([P, H], WDT, tag="w1k", name=f"w1k{k}")
            nc.sync.dma_start(out=wt, in_=w1[k])
            last = k == L1_NK - 1
            for ob in range(8):
                nc.tensor.matmul(
                    pb[ob],
                    wt[:, ob * P : (ob + 1) * P],
                    rhs_ap,
                    start=(k == 0),
                    stop=last,
                )

        for k in range(4):  # silu base path
            l1_block(k, si1[k])
        for m in range(NM):
            for ib in range(4):
                r = _emit_features(nc, tmp, rp, u1[ib], m, RDT, sq_on_act=True,
                                   name=f"a{ib}", bias_ap=bias_ap)
                l1_block(4 + m * 4 + ib, r)

        # ---- boundary: h = out1 lives in PSUM [h_blk, tok]; derive L2 feats
        u2, si2 = [], []
        for j in range(8):
            t1b = tmp.tile([P, TOK], F32, tag="t1", name=f"t1b_{j}")
            nc.scalar.activation(t1b, pb[j], AF.Relu, bias=bias_ap(5.5), scale=2.5)
            s = persist.tile([P, TOK], RDT, tag=f"si2{j}", name=f"si2_{j}")
            nc.scalar.activation(s, pb[j], AF.Silu, bias=bias_ap(0.0))
            si2.append(s)
            u = persist.tile([P, TOK], F32, tag=f"u2{j}", name=f"u2_{j}")
            nc.vector.tensor_scalar_min(u, t1b, UMAX)
            u2.append(u)

        # ---- L2 matmuls: out2[tok_blk, o] over 96 K-tiles ---------------
        qb = [
            psum.tile([P, O], F32, tag=f"p{tb}", name=f"q{tb}") for tb in range(4)
        ]

        def l2_block(k, lhsT_tile):
            wt = w2p.tile([P, O], WDT, tag="w2k", name=f"w2k{k}")
            nc.sync.dma_start(out=wt, in_=w2[k])
            last = k == L2_NK - 1
            for tb in range(4):
                nc.tensor.matmul(
                    qb[tb],
                    lhsT_tile[:, tb * P : (tb + 1) * P],
                    wt,
                    start=(k == 0),
                    stop=last,
                )

        for k in range(8):  # silu base path
            l2_block(k, si2[k])
        for m in range(NM):
            for j in range(8):
                r = _emit_features(nc, tmp, rp, u2[j], m, RDT,
                                   sq_on_act=(m in L2_SQ_ACT_M), name=f"b{j}", bias_ap=bias_ap)
                l2_block(8 + m * 8 + j, r)

        # ---- store ------------------------------------------------------
        for tb in range(4):
            ot = outp.tile([P, O], F32, tag="ot", name=f"ot{tb}")
            nc.vector.tensor_copy(ot, qb[tb])
            nc.sync.dma_start(out=out[tb * P : (tb + 1) * P, :], in_=ot)

    nc.compile()
    _BUILD_CACHE["nc"] = nc
    return nc


# ---------------------------------------------------------------- entry
def kernel(x, coef1, scale_base1, scale_sp1, coef2, scale_base2, scale_sp2,
           _want_trace=False):
    from concourse.bass_utils import run_bass_kernel_spmd

    wdt = _np_wdt()
    x_flat = np.asarray(x, np.float32).reshape(NTOK, D)
    w1 = _pack_w1(np.asarray(coef1), np.asarray(scale_sp1), np.asarray(scale_base1))
    w2 = _pack_w2(np.asarray(coef2), np.asarray(scale_sp2), np.asarray(scale_base2))

    nc = _build_kernel()

    in_maps = []
    for c in range(NCORES):
        xs = x_flat[c * TOK : (c + 1) * TOK]  # (TOK, D)
        in_maps.append(
            {
                "xT": np.ascontiguousarray(xs.T),
                "w1": w1,
                "w2": w2,
            }
        )

    res = run_bass_kernel_spmd(
        nc, in_maps, core_ids=list(range(NCORES)), trace=_want_trace
    )
    outs = [res.results[c]["out"] for c in range(NCORES)]
    full = np.concatenate(outs, axis=0).reshape(x.shape[0], x.shape[1], O)
    if _want_trace:
        kernel._last_results = res  # stash for test harness profiling
    return full.astype(np.float32)



# revision 5
# speedup vs baseline: 3.4221x; 3.4221x over previous
"""KAN feed-forward on Trainium2 — Bass/Tile kernel, 8-core data-parallel.

Math: each KAN layer is
    y = silu(x) @ scale_base + sum_g B_g(u) @ (coef[:,:,g]*scale_sp)/6,
with u = 2.5 x + 5.5 and B_g the (G=5, K=3) uniform-grid cubic B-spline basis,
B_g(u) = b3(u - g) (cardinal b3, support [0,4]).  Direct-basis evaluation:
    6*b3(u-g) = relu(2 - a)^3 - 4*relu(1 - a)^3,   a = |u - (g+2)|,
computed by two fused custom-DVE passes per basis tile (PCUBE then QCUBE_ACC).
All basis values vanish for u <= 0 / >= 11, so relu(2.5x+5.5) needs no upper
clamp and matches the reference exactly for saturated x.

Why direct basis: features are bounded in [0,4] (vs truncated powers up to
1331), so there is no catastrophic cancellation in the matmul and fp16
operands pass the 2e-2 gate with ~20x margin (measured 1e-3).  fp16 streams
the PE at 1 cycle/row (4x over fp32), and the K-dim shrinks from 12 to 9
tiles per input block (8 basis + 1 silu vs 11 powers + 1 silu).

Per-core layout (512 tokens/core):
  L1: out1[o, tok] (+=) over 36 K-tiles (4 silu + 32 basis), lhsT = W1 tiles,
      rhs = f16 feature tiles [128, 512] built from xT by ACT/DVE.
  L2: out2[tok, o] over 72 K-tiles (8 silu + 64 basis), lhsT = feature slices,
      rhs = W2 tiles.  L1's PSUM output [h, tok] is exactly the transposed
      layout L2's feature construction needs — no transposes.
"""

import sys
from contextlib import ExitStack

import numpy as np

for _p in ("/opt/trn_rl_repo",):
    if _p not in sys.path:
        sys.path.insert(0, _p)

# ---------------------------------------------------------------- constants
NB = 8  # basis functions per input dim
D, H, O = 512, 1024, 512
NCORES = 8
NTOK = 4096
TOK = NTOK // NCORES  # 512 tokens per core
P = 128

L1_NK = 4 + NB * 4  # 36 K-tiles of 128 (4 silu + 32 basis)
L2_NK = 8 + NB * 8  # 72 K-tiles of 128 (8 silu + 64 basis)

_BUILD_CACHE: dict = {}
_DVE_OPS_CACHE: dict = {}


# ------------------------------------------------------- custom DVE ops
def _register_dve_ops():
    """Register the two fused basis ops in dve_ops.OPS (documented authoring
    path: define a DveOp and append to OPS; the uop program is written into
    the per-NEFF DVE table at compile time)."""
    if _DVE_OPS_CACHE:
        return _DVE_OPS_CACHE["P"], _DVE_OPS_CACHE["Q"]

    from concourse import dve_ops
    from concourse.dve_spec import (
        AluOp, Bin, C0, C1, C2, Spec, Src0, Src1, _has_src1, lower, relu, sq,
    )
    from concourse.dve_uop import DveOpSpec

    def absdiff(x, c):
        return Bin(AluOp.ABSOLUTE_DIFF, x, c)

    # PCUBE: out = relu(C1 - |x - C0|)^3
    pA = relu(C1 - absdiff(Src0, C0))
    specA = Spec(
        body=sq(pA) * pA,
        reference=lambda in0, in1, s0, s1, imm2: np.maximum(
            s1 - np.abs(in0 - s0), 0.0
        ) ** 3,
    )
    # QCUBE_ACC: out = Src1 + C2 * relu(C1 - |x - C0|)^3
    qB = relu(C1 - absdiff(Src0, C0))
    specB = Spec(
        body=Src1 + sq(qB) * qB * C2,
        reference=lambda in0, in1, s0, s1, imm2: in1
        + imm2 * np.maximum(s1 - np.abs(in0 - s0), 0.0) ** 3,
    )

    def mk(name, spec):
        for op in dve_ops.OPS:  # already registered (e.g. by a prior import)
            if op.name == name:
                return op
        row = dve_ops._CUSTOM_DVE_ROW_BASE + len(dve_ops.OPS)
        assert row < 0x20, "DVE opcode rows exhausted"
        shas = {}
        for ver in ("v3", "v4"):
            s = DveOpSpec(
                name=name, opcode=row, uops=lower(spec, ver=ver),
                rd1_en=_has_src1(spec),
            )
            shas[ver] = s.sha(ver)
        op = dve_ops.DveOp(name, spec, subdim=False, uops_sha=shas)
        dve_ops.OPS.append(op)
        dve_ops._SUB_OPCODE_FOR_NAME[name] = row
        dve_ops.CUSTOM_DVE_SPECS[name] = spec
        return op

    opP = mk("BSPL_PCUBE_ANT", specA)
    opQ = mk("BSPL_QCUBE_ACC_ANT", specB)
    _DVE_OPS_CACHE["P"] = opP
    _DVE_OPS_CACHE["Q"] = opQ
    return opP, opQ


# ---------------------------------------------------------------- host prep
def _pack_w1(coef1, scale_sp1, scale_base1) -> np.ndarray:
    """-> (36, 128, 1024) f16: [4 silu blocks, then g-major basis blocks]."""
    A = coef1.astype(np.float64) * scale_sp1.astype(np.float64)[:, :, None] / 6.0
    w1 = np.empty((L1_NK, P, H), np.float16)
    for ib in range(4):
        w1[ib] = scale_base1[ib * P : (ib + 1) * P].astype(np.float16)
    for g in range(NB):
        for ib in range(4):
            w1[4 + g * 4 + ib] = A[ib * P : (ib + 1) * P, :, g].astype(np.float16)
    return np.ascontiguousarray(w1)


def _pack_w2(coef2, scale_sp2, scale_base2) -> np.ndarray:
    """-> (72, 128, 512) f16."""
    A = coef2.astype(np.float64) * scale_sp2.astype(np.float64)[:, :, None] / 6.0
    w2 = np.empty((L2_NK, P, O), np.float16)
    for j in range(8):
        w2[j] = scale_base2[j * P : (j + 1) * P].astype(np.float16)
    for g in range(NB):
        for j in range(8):
            w2[8 + g * 8 + j] = A[j * P : (j + 1) * P, :, g].astype(np.float16)
    return np.ascontiguousarray(w2)


# ---------------------------------------------------------------- bass build
def _build_kernel():
    """Build + compile the Bass program once; cached per process."""
    if "nc" in _BUILD_CACHE:
        return _BUILD_CACHE["nc"]

    import concourse.mybir as mybir
    import concourse.tile as tile
    from concourse import bacc

    opP, opQ = _register_dve_ops()

    AF = mybir.ActivationFunctionType
    F32 = mybir.dt.float32
    F16 = mybir.dt.float16

    nc = bacc.Bacc("TRN2", target_bir_lowering=False, debug=False, num_devices=NCORES)

    xT = nc.dram_tensor("xT", (D, TOK), F32, kind="ExternalInput").ap()
    w1 = nc.dram_tensor("w1", (L1_NK, P, H), F16, kind="ExternalInput").ap()
    w2 = nc.dram_tensor("w2", (L2_NK, P, O), F16, kind="ExternalInput").ap()
    out = nc.dram_tensor("out", (TOK, O), F32, kind="ExternalOutput").ap()

    with tile.TileContext(nc) as tc, ExitStack() as ctx:
        persist = ctx.enter_context(tc.tile_pool(name="persist", bufs=1))
        tmp = ctx.enter_context(tc.tile_pool(name="tmp", bufs=3))
        rp = ctx.enter_context(tc.tile_pool(name="rp", bufs=4))
        w1p = ctx.enter_context(tc.tile_pool(name="w1p", bufs=4))
        w2p = ctx.enter_context(tc.tile_pool(name="w2p", bufs=6))
        outp = ctx.enter_context(tc.tile_pool(name="outp", bufs=4))
        psum = ctx.enter_context(tc.tile_pool(name="psum", bufs=1, space="PSUM"))

        _bias_cache: dict = {}

        def bias_ap(val: float):
            if val not in _bias_cache:
                t = persist.tile([P, 1], F32, tag=f"bias{len(_bias_cache)}",
                                 name=f"bias_{len(_bias_cache)}")
                nc.vector.memset(t, val)
                _bias_cache[val] = t
            return _bias_cache[val]

        def emit_basis(u_tile, g, name):
            """F = relu(2-a)^3 - 4*relu(1-a)^3, a = |u - (g+2)|; f16 out."""
            c = float(g + 2)
            t = tmp.tile([P, TOK], F32, tag="pc", name=f"pc{name}g{g}")
            nc.vector._custom_dve(opP, out=t, in0=u_tile, s0=c, s1=2.0)
            f = rp.tile([P, TOK], F16, tag="qf", name=f"qf{name}g{g}")
            nc.vector._custom_dve(opQ, out=f, in0=u_tile, in1=t, s0=c, s1=1.0,
                                  imm2=-4.0)
            return f

        # ---- L1 inputs: xT tiles + activations --------------------------
        xt = []
        for ib in range(4):
            t = persist.tile([P, TOK], F32, tag="xt", bufs=2, name=f"xt{ib}")
            nc.sync.dma_start(out=t, in_=xT[ib * P : (ib + 1) * P, :])
            xt.append(t)

        u1, si1 = [], []
        for ib in range(4):
            u = persist.tile([P, TOK], F32, tag=f"u1{ib}", name=f"u1_{ib}")
            nc.scalar.activation(u, xt[ib], AF.Relu, bias=bias_ap(5.5), scale=2.5)
            u1.append(u)
            s = persist.tile([P, TOK], F16, tag=f"si1{ib}", name=f"si1_{ib}")
            nc.scalar.activation(s, xt[ib], AF.Silu, bias=bias_ap(0.0))
            si1.append(s)

        # ---- L1 matmuls: out1[o_blk, tok] accumulated over 36 K-tiles ---
        pb = [
            psum.tile([P, TOK], F32, tag=f"p{ob}", name=f"p{ob}") for ob in range(8)
        ]

        def l1_block(k, rhs_ap):
            wt = w1p.tile([P, H], F16, tag="w1k", name=f"w1k{k}")
            nc.sync.dma_start(out=wt, in_=w1[k])
            last = k == L1_NK - 1
            for ob in range(8):
                nc.tensor.matmul(
                    pb[ob],
                    wt[:, ob * P : (ob + 1) * P],
                    rhs_ap,
                    start=(k == 0),
                    stop=last,
                )

        for k in range(4):  # silu base path
            l1_block(k, si1[k])
        for g in range(NB):
            for ib in range(4):
                l1_block(4 + g * 4 + ib, emit_basis(u1[ib], g, f"a{ib}"))

        # ---- boundary: h = out1 lives in PSUM [h_blk, tok]; L2 inputs ---
        u2, si2 = [], []
        for j in range(8):
            u = persist.tile([P, TOK], F32, tag=f"u2{j}", name=f"u2_{j}")
            nc.scalar.activation(u, pb[j], AF.Relu, bias=bias_ap(5.5), scale=2.5)
            u2.append(u)
            s = persist.tile([P, TOK], F16, tag=f"si2{j}", name=f"si2_{j}")
            nc.scalar.activation(s, pb[j], AF.Silu, bias=bias_ap(0.0))
            si2.append(s)

        # ---- L2 matmuls: out2[tok_blk, o] over 72 K-tiles ---------------
        qb = [
            psum.tile([P, O], F32, tag=f"p{tb}", name=f"q{tb}") for tb in range(4)
        ]

        def l2_block(k, lhsT_tile):
            wt = w2p.tile([P, O], F16, tag="w2k", name=f"w2k{k}")
            nc.sync.dma_start(out=wt, in_=w2[k])
            last = k == L2_NK - 1
            for tb in range(4):
                nc.tensor.matmul(
                    qb[tb],
                    lhsT_tile[:, tb * P : (tb + 1) * P],
                    wt,
                    start=(k == 0),
                    stop=last,
                )

        for k in range(8):  # silu base path
            l2_block(k, si2[k])
        for g in range(NB):
            for j in range(8):
                l2_block(8 + g * 8 + j, emit_basis(u2[j], g, f"b{j}"))

        # ---- store ------------------------------------------------------
        for tb in range(4):
            ot = outp.tile([P, O], F32, tag="ot", name=f"ot{tb}")
            nc.vector.tensor_copy(ot, qb[tb])
            nc.sync.dma_start(out=out[tb * P : (tb + 1) * P, :], in_=ot)

    nc.compile()
    _BUILD_CACHE["nc"] = nc
    return nc


# ---------------------------------------------------------------- entry
def kernel(x, coef1, scale_base1, scale_sp1, coef2, scale_base2, scale_sp2,
           _want_trace=False):
    from concourse.bass_utils import run_bass_kernel_spmd

    x_flat = np.asarray(x, np.float32).reshape(NTOK, D)
    w1 = _pack_w1(np.asarray(coef1), np.asarray(scale_sp1), np.asarray(scale_base1))
    w2 = _pack_w2(np.asarray(coef2), np.asarray(scale_sp2), np.asarray(scale_base2))

    nc = _build_kernel()

    in_maps = []
    for c in range(NCORES):
        xs = x_flat[c * TOK : (c + 1) * TOK]  # (TOK, D)
        in_maps.append(
            {
                "xT": np.ascontiguousarray(xs.T),
                "w1": w1,
                "w2": w2,
            }
        )

    res = run_bass_kernel_spmd(
        nc, in_maps, core_ids=list(range(NCORES)), trace=_want_trace
    )
    outs = [res.results[c]["out"] for c in range(NCORES)]
    full = np.concatenate(outs, axis=0).reshape(x.shape[0], x.shape[1], O)
    if _want_trace:
        kernel._last_results = res  # stash for test harness profiling
    return full.astype(np.float32)


# revision 6
# speedup vs baseline: 3.9258x; 1.1472x over previous
"""KAN feed-forward on Trainium2 — Bass/Tile kernel, 8-core data-parallel.

Math: each KAN layer is
    y = silu(x) @ scale_base + sum_g B_g(u) @ (coef[:,:,g]*scale_sp)/6,
with u = 2.5 x + 5.5 and B_g the (G=5, K=3) uniform-grid cubic B-spline basis,
B_g(u) = b3(u - g) (cardinal b3, support [0,4]).  Direct-basis evaluation:
    6*b3(u-g) = relu(2 - a)^3 - 4*relu(1 - a)^3,   a = |u - (g+2)|,
computed by two fused custom-DVE passes per basis feature (PCUBE then
QCUBE_ACC).  All basis values vanish for u <= 0 / >= 11, so relu(2.5x+5.5)
needs no upper clamp and matches the reference exactly for saturated x.

Why direct basis: features are bounded in [0,4] (vs truncated powers up to
1331), so there is no catastrophic cancellation in the matmul and fp16
operands pass the 2e-2 gate with ~20x margin (measured ~1e-3).  fp16 streams
the PE at 1 cycle/row (4x over fp32), and the K-dim shrinks from 12 to 9
tiles per input block (8 basis + 1 silu).

Engine balance (trace-driven): each DVE instruction pays ~300 cycles of fixed
overhead, so basis features are built in mega-tiles spanning all input blocks
in the free dim (L1: [128, 2048], L2: [128, 4096]) — 2 DVE passes per basis
function per layer instead of 2 per (function, block).  The u = 2.5x+5.5
affine prep runs on DVE (tensor_scalar, idle at those points); silu runs on
ACT.  L1 is PE-bound (~75us), L2 is PE/DVE-balanced (~75us each).

Per-core layout (512 tokens/core):
  L1: out1[o, tok] (+=) over 36 K-tiles (4 silu + 32 basis), lhsT = W1 tiles,
      rhs = f16 feature slices of the L1 mega-tiles.
  L2: out2[tok, o] over 72 K-tiles, lhsT = feature slices, rhs = W2 tiles.
      L1's PSUM output [h_blk, tok] is exactly the transposed layout L2's
      feature construction needs — no transposes.
"""

import sys
from contextlib import ExitStack

import numpy as np

for _p in ("/opt/trn_rl_repo",):
    if _p not in sys.path:
        sys.path.insert(0, _p)

# ---------------------------------------------------------------- constants
NB = 8  # basis functions per input dim
D, H, O = 512, 1024, 512
NCORES = 8
NTOK = 4096
TOK = NTOK // NCORES  # 512 tokens per core
P = 128

L1_NK = 4 + NB * 4  # 36 K-tiles of 128 (4 silu + 32 basis)
L2_NK = 8 + NB * 8  # 72 K-tiles of 128 (8 silu + 64 basis)

_BUILD_CACHE: dict = {}
_DVE_OPS_CACHE: dict = {}


# ------------------------------------------------------- custom DVE ops
def _register_dve_ops():
    """Register the two fused basis ops in dve_ops.OPS (documented authoring
    path: define a DveOp and append to OPS; the uop program is written into
    the per-NEFF DVE table at compile time)."""
    if _DVE_OPS_CACHE:
        return _DVE_OPS_CACHE["P"], _DVE_OPS_CACHE["Q"]

    from concourse import dve_ops
    from concourse.dve_spec import (
        AluOp, Bin, C0, C1, C2, Spec, Src0, Src1, _has_src1, lower, relu, sq,
    )
    from concourse.dve_uop import DveOpSpec

    def absdiff(x, c):
        return Bin(AluOp.ABSOLUTE_DIFF, x, c)

    # PCUBE: out = relu(C1 - |x - C0|)^3
    pA = relu(C1 - absdiff(Src0, C0))
    specA = Spec(
        body=sq(pA) * pA,
        reference=lambda in0, in1, s0, s1, imm2: np.maximum(
            s1 - np.abs(in0 - s0), 0.0
        ) ** 3,
    )
    # QCUBE_ACC: out = Src1 + C2 * relu(C1 - |x - C0|)^3
    qB = relu(C1 - absdiff(Src0, C0))
    specB = Spec(
        body=Src1 + sq(qB) * qB * C2,
        reference=lambda in0, in1, s0, s1, imm2: in1
        + imm2 * np.maximum(s1 - np.abs(in0 - s0), 0.0) ** 3,
    )

    def mk(name, spec):
        for op in dve_ops.OPS:  # already registered (e.g. by a prior import)
            if op.name == name:
                return op
        row = dve_ops._CUSTOM_DVE_ROW_BASE + len(dve_ops.OPS)
        assert row < 0x20, "DVE opcode rows exhausted"
        shas = {}
        for ver in ("v3", "v4"):
            s = DveOpSpec(
                name=name, opcode=row, uops=lower(spec, ver=ver),
                rd1_en=_has_src1(spec),
            )
            shas[ver] = s.sha(ver)
        op = dve_ops.DveOp(name, spec, subdim=False, uops_sha=shas)
        dve_ops.OPS.append(op)
        dve_ops._SUB_OPCODE_FOR_NAME[name] = row
        dve_ops.CUSTOM_DVE_SPECS[name] = spec
        return op

    opP = mk("BSPL_PCUBE_ANT", specA)
    opQ = mk("BSPL_QCUBE_ACC_ANT", specB)
    _DVE_OPS_CACHE["P"] = opP
    _DVE_OPS_CACHE["Q"] = opQ
    return opP, opQ


# ---------------------------------------------------------------- host prep
def _pack_w1(coef1, scale_sp1, scale_base1) -> np.ndarray:
    """-> (36, 128, 1024) f16: [4 silu blocks, then g-major basis blocks]."""
    A = coef1.astype(np.float64) * scale_sp1.astype(np.float64)[:, :, None] / 6.0
    w1 = np.empty((L1_NK, P, H), np.float16)
    for ib in range(4):
        w1[ib] = scale_base1[ib * P : (ib + 1) * P].astype(np.float16)
    for g in range(NB):
        for ib in range(4):
            w1[4 + g * 4 + ib] = A[ib * P : (ib + 1) * P, :, g].astype(np.float16)
    return np.ascontiguousarray(w1)


def _pack_w2(coef2, scale_sp2, scale_base2) -> np.ndarray:
    """-> (72, 128, 512) f16."""
    A = coef2.astype(np.float64) * scale_sp2.astype(np.float64)[:, :, None] / 6.0
    w2 = np.empty((L2_NK, P, O), np.float16)
    for j in range(8):
        w2[j] = scale_base2[j * P : (j + 1) * P].astype(np.float16)
    for g in range(NB):
        for j in range(8):
            w2[8 + g * 8 + j] = A[j * P : (j + 1) * P, :, g].astype(np.float16)
    return np.ascontiguousarray(w2)


# ---------------------------------------------------------------- bass build
def _build_kernel():
    """Build + compile the Bass program once; cached per process."""
    if "nc" in _BUILD_CACHE:
        return _BUILD_CACHE["nc"]

    import concourse.mybir as mybir
    import concourse.tile as tile
    from concourse import bacc

    opP, opQ = _register_dve_ops()

    AF = mybir.ActivationFunctionType
    ALU = mybir.AluOpType
    F32 = mybir.dt.float32
    F16 = mybir.dt.float16

    nc = bacc.Bacc("TRN2", target_bir_lowering=False, debug=False, num_devices=NCORES)

    xT = nc.dram_tensor("xT", (D, TOK), F32, kind="ExternalInput").ap()
    w1 = nc.dram_tensor("w1", (L1_NK, P, H), F16, kind="ExternalInput").ap()
    w2 = nc.dram_tensor("w2", (L2_NK, P, O), F16, kind="ExternalInput").ap()
    out = nc.dram_tensor("out", (TOK, O), F32, kind="ExternalOutput").ap()

    W1FREE = 4 * TOK  # L1 mega-tile free dim (all 4 input blocks)
    W2FREE = 8 * TOK  # L2 mega-tile free dim (all 8 hidden blocks)

    with tile.TileContext(nc) as tc, ExitStack() as ctx:
        persist = ctx.enter_context(tc.tile_pool(name="persist", bufs=1))
        tmp = ctx.enter_context(tc.tile_pool(name="tmp", bufs=2))
        rp = ctx.enter_context(tc.tile_pool(name="rp", bufs=3))
        w1p = ctx.enter_context(tc.tile_pool(name="w1p", bufs=4))
        w2p = ctx.enter_context(tc.tile_pool(name="w2p", bufs=6))
        outp = ctx.enter_context(tc.tile_pool(name="outp", bufs=4))
        psum = ctx.enter_context(tc.tile_pool(name="psum", bufs=1, space="PSUM"))

        _bias_cache: dict = {}

        def bias_ap(val: float):
            if val not in _bias_cache:
                t = persist.tile([P, 1], F32, tag=f"bias{len(_bias_cache)}",
                                 name=f"bias_{len(_bias_cache)}")
                nc.vector.memset(t, val)
                _bias_cache[val] = t
            return _bias_cache[val]

        def emit_basis(u_mega, g, free, name):
            """F = relu(2-a)^3 - 4*relu(1-a)^3, a = |u - (g+2)|; f16 mega."""
            c = float(g + 2)
            t = tmp.tile([P, free], F16, tag=f"pc{name}", name=f"pc{name}g{g}")
            nc.vector._custom_dve(opP, out=t, in0=u_mega, s0=c, s1=2.0)
            f = rp.tile([P, free], F16, tag=f"qf{name}", name=f"qf{name}g{g}")
            nc.vector._custom_dve(opQ, out=f, in0=u_mega, in1=t, s0=c, s1=1.0,
                                  imm2=-4.0)
            return f

        # ---- L1 inputs: x mega-tile + activations -----------------------
        xm = persist.tile([P, W1FREE], F32, tag="xm", name="xm")
        for ib in range(4):
            nc.sync.dma_start(
                out=xm[:, ib * TOK : (ib + 1) * TOK],
                in_=xT[ib * P : (ib + 1) * P, :],
            )

        u1 = persist.tile([P, W1FREE], F32, tag="u1", name="u1")
        nc.vector.tensor_scalar(u1, xm, 2.5, 5.5, ALU.mult, ALU.add)
        si1 = persist.tile([P, W1FREE], F16, tag="si1", name="si1")
        nc.scalar.activation(si1, xm, AF.Silu, bias=bias_ap(0.0))

        # ---- L1 matmuls: out1[o_blk, tok] accumulated over 36 K-tiles ---
        pb = [
            psum.tile([P, TOK], F32, tag=f"p{ob}", name=f"p{ob}") for ob in range(8)
        ]

        def l1_block(k, rhs_ap):
            wt = w1p.tile([P, H], F16, tag="w1k", name=f"w1k{k}")
            nc.sync.dma_start(out=wt, in_=w1[k])
            last = k == L1_NK - 1
            for ob in range(8):
                nc.tensor.matmul(
                    pb[ob],
                    wt[:, ob * P : (ob + 1) * P],
                    rhs_ap,
                    start=(k == 0),
                    stop=last,
                )

        for k in range(4):  # silu base path
            l1_block(k, si1[:, k * TOK : (k + 1) * TOK])
        for g in range(NB):
            f = emit_basis(u1, g, W1FREE, "a")
            for ib in range(4):
                l1_block(4 + g * 4 + ib, f[:, ib * TOK : (ib + 1) * TOK])

        # ---- boundary: h = out1 in PSUM [h_blk, tok]; L2 inputs ---------
        # silu on ACT, u-affine on DVE, interleaved so PSUM banks free early.
        u2 = persist.tile([P, W2FREE], F32, tag="u2", name="u2")
        si2 = persist.tile([P, W2FREE], F16, tag="si2", name="si2")
        for j in range(8):
            nc.scalar.activation(si2[:, j * TOK : (j + 1) * TOK], pb[j],
                                 AF.Silu, bias=bias_ap(0.0))
            nc.vector.tensor_scalar(u2[:, j * TOK : (j + 1) * TOK], pb[j],
                                    2.5, 5.5, ALU.mult, ALU.add)

        # ---- L2 matmuls: out2[tok_blk, o] over 72 K-tiles ---------------
        qb = [
            psum.tile([P, O], F32, tag=f"p{tb}", name=f"q{tb}") for tb in range(4)
        ]

        def l2_block(k, lhsT_ap):
            wt = w2p.tile([P, O], F16, tag="w2k", name=f"w2k{k}")
            nc.sync.dma_start(out=wt, in_=w2[k])
            last = k == L2_NK - 1
            for tb in range(4):
                nc.tensor.matmul(
                    qb[tb],
                    lhsT_ap[:, tb * P : (tb + 1) * P],
                    wt,
                    start=(k == 0),
                    stop=last,
                )

        for k in range(8):  # silu base path
            l2_block(k, si2[:, k * TOK : (k + 1) * TOK])
        for g in range(NB):
            f = emit_basis(u2, g, W2FREE, "b")
            for j in range(8):
                l2_block(8 + g * 8 + j, f[:, j * TOK : (j + 1) * TOK])

        # ---- store ------------------------------------------------------
        for tb in range(4):
            ot = outp.tile([P, O], F32, tag="ot", name=f"ot{tb}")
            nc.vector.tensor_copy(ot, qb[tb])
            nc.sync.dma_start(out=out[tb * P : (tb + 1) * P, :], in_=ot)

    nc.compile()
    _BUILD_CACHE["nc"] = nc
    return nc


# ---------------------------------------------------------------- entry
def kernel(x, coef1, scale_base1, scale_sp1, coef2, scale_base2, scale_sp2,
           _want_trace=False):
    from concourse.bass_utils import run_bass_kernel_spmd

    x_flat = np.asarray(x, np.float32).reshape(NTOK, D)
    w1 = _pack_w1(np.asarray(coef1), np.asarray(scale_sp1), np.asarray(scale_base1))
    w2 = _pack_w2(np.asarray(coef2), np.asarray(scale_sp2), np.asarray(scale_base2))

    nc = _build_kernel()

    in_maps = []
    for c in range(NCORES):
        xs = x_flat[c * TOK : (c + 1) * TOK]  # (TOK, D)
        in_maps.append(
            {
                "xT": np.ascontiguousarray(xs.T),
                "w1": w1,
                "w2": w2,
            }
        )

    res = run_bass_kernel_spmd(
        nc, in_maps, core_ids=list(range(NCORES)), trace=_want_trace
    )
    outs = [res.results[c]["out"] for c in range(NCORES)]
    full = np.concatenate(outs, axis=0).reshape(x.shape[0], x.shape[1], O)
    if _want_trace:
        kernel._last_results = res  # stash for test harness profiling
    return full.astype(np.float32)


# revision 9
# speedup vs baseline: 4.0144x; 1.0226x over previous
"""KAN feed-forward on Trainium2 — Bass/Tile kernel, 8-core data-parallel.

Math: each KAN layer is
    y = silu(x) @ scale_base + sum_g B_g(u) @ (coef[:,:,g]*scale_sp)/6,
with u = 2.5 x + 5.5 and B_g the (G=5, K=3) uniform-grid cubic B-spline basis,
B_g(u) = b3(u - g) (cardinal b3, support [0,4]).  Direct-basis evaluation:
    6*b3(u-g) = relu(2 - a)^3 - 4*relu(1 - a)^3,   a = |u - (g+2)|,
computed by two fused custom-DVE passes per basis feature (PCUBE then
QCUBE_ACC).  All basis values vanish for u <= 0 / >= 11, so relu(2.5x+5.5)
needs no upper clamp and matches the reference exactly for saturated x.

Why direct basis: features are bounded in [0,4] (vs truncated powers up to
1331), so there is no catastrophic cancellation in the matmul and fp16
operands pass the 2e-2 gate with ~20x margin (measured ~1e-3).  fp16 streams
the PE at 1 cycle/row (4x over fp32), and the K-dim shrinks from 12 to 9
tiles per input block (8 basis + 1 silu).

Engine balance (trace-driven): each DVE instruction pays ~300 cycles of fixed
overhead, so basis features are built in mega-tiles spanning all input blocks
in the free dim (L1: [128, 2048], L2: [128, 4096]) — 2 DVE passes per basis
function per layer instead of 2 per (function, block).  The u = 2.5x+5.5
affine prep runs on DVE (tensor_scalar, idle at those points); silu runs on
ACT.  L1 is PE-bound (~75us), L2 is PE/DVE-balanced (~75us each).

Per-core layout (512 tokens/core):
  L1: out1[o, tok] (+=) over 36 K-tiles (4 silu + 32 basis), lhsT = W1 tiles,
      rhs = f16 feature slices of the L1 mega-tiles.
  L2: out2[tok, o] over 72 K-tiles, lhsT = feature slices, rhs = W2 tiles.
      L1's PSUM output [h_blk, tok] is exactly the transposed layout L2's
      feature construction needs — no transposes.
"""

import sys
from contextlib import ExitStack

import numpy as np

for _p in ("/opt/trn_rl_repo",):
    if _p not in sys.path:
        sys.path.insert(0, _p)

# ---------------------------------------------------------------- constants
NB = 8  # basis functions per input dim
D, H, O = 512, 1024, 512
NCORES = 8
NTOK = 4096
TOK = NTOK // NCORES  # 512 tokens per core
P = 128

L1_NK = 4 + NB * 4  # 36 K-tiles of 128 (4 silu + 32 basis)
L2_NK = 8 + NB * 8  # 72 K-tiles of 128 (8 silu + 64 basis)

_BUILD_CACHE: dict = {}
_DVE_OPS_CACHE: dict = {}


# ------------------------------------------------------- custom DVE ops
def _register_dve_ops():
    """Register the two fused basis ops in dve_ops.OPS (documented authoring
    path: define a DveOp and append to OPS; the uop program is written into
    the per-NEFF DVE table at compile time)."""
    if _DVE_OPS_CACHE:
        return _DVE_OPS_CACHE["P"], _DVE_OPS_CACHE["Q"]

    from concourse import dve_ops
    from concourse.dve_spec import (
        AluOp, Bin, C0, C1, C2, Spec, Src0, Src1, _has_src1, lower, relu, sq,
    )
    from concourse.dve_uop import DveOpSpec

    def absdiff(x, c):
        return Bin(AluOp.ABSOLUTE_DIFF, x, c)

    # PCUBE: out = relu(C1 - |x - C0|)^3
    pA = relu(C1 - absdiff(Src0, C0))
    specA = Spec(
        body=sq(pA) * pA,
        reference=lambda in0, in1, s0, s1, imm2: np.maximum(
            s1 - np.abs(in0 - s0), 0.0
        ) ** 3,
    )
    # QCUBE_ACC: out = Src1 + C2 * relu(C1 - |x - C0|)^3
    qB = relu(C1 - absdiff(Src0, C0))
    specB = Spec(
        body=Src1 + sq(qB) * qB * C2,
        reference=lambda in0, in1, s0, s1, imm2: in1
        + imm2 * np.maximum(s1 - np.abs(in0 - s0), 0.0) ** 3,
    )

    def mk(name, spec):
        for op in dve_ops.OPS:  # already registered (e.g. by a prior import)
            if op.name == name:
                return op
        row = dve_ops._CUSTOM_DVE_ROW_BASE + len(dve_ops.OPS)
        assert row < 0x20, "DVE opcode rows exhausted"
        shas = {}
        for ver in ("v3", "v4"):
            s = DveOpSpec(
                name=name, opcode=row, uops=lower(spec, ver=ver),
                rd1_en=_has_src1(spec),
            )
            shas[ver] = s.sha(ver)
        op = dve_ops.DveOp(name, spec, subdim=False, uops_sha=shas)
        dve_ops.OPS.append(op)
        dve_ops._SUB_OPCODE_FOR_NAME[name] = row
        dve_ops.CUSTOM_DVE_SPECS[name] = spec
        return op

    opP = mk("BSPL_PCUBE_ANT", specA)
    opQ = mk("BSPL_QCUBE_ACC_ANT", specB)
    _DVE_OPS_CACHE["P"] = opP
    _DVE_OPS_CACHE["Q"] = opQ
    return opP, opQ


# ---------------------------------------------------------------- host prep
def _pack_w1(coef1, scale_sp1, scale_base1) -> np.ndarray:
    """-> (36, 128, 1024) f16: [4 silu blocks, then g-major basis blocks]."""
    A = coef1.astype(np.float64) * scale_sp1.astype(np.float64)[:, :, None] / 6.0
    w1 = np.empty((L1_NK, P, H), np.float16)
    for ib in range(4):
        w1[ib] = scale_base1[ib * P : (ib + 1) * P].astype(np.float16)
    for g in range(NB):
        for ib in range(4):
            w1[4 + g * 4 + ib] = A[ib * P : (ib + 1) * P, :, g].astype(np.float16)
    return np.ascontiguousarray(w1)


def _pack_w2(coef2, scale_sp2, scale_base2) -> np.ndarray:
    """-> (72, 128, 512) f16."""
    A = coef2.astype(np.float64) * scale_sp2.astype(np.float64)[:, :, None] / 6.0
    w2 = np.empty((L2_NK, P, O), np.float16)
    for j in range(8):
        w2[j] = scale_base2[j * P : (j + 1) * P].astype(np.float16)
    for g in range(NB):
        for j in range(8):
            w2[8 + g * 8 + j] = A[j * P : (j + 1) * P, :, g].astype(np.float16)
    return np.ascontiguousarray(w2)


# ---------------------------------------------------------------- bass build
def _build_kernel():
    """Build + compile the Bass program once; cached per process."""
    if "nc" in _BUILD_CACHE:
        return _BUILD_CACHE["nc"]

    import concourse.mybir as mybir
    import concourse.tile as tile
    from concourse import bacc

    opP, opQ = _register_dve_ops()

    AF = mybir.ActivationFunctionType
    ALU = mybir.AluOpType
    F32 = mybir.dt.float32
    F16 = mybir.dt.float16

    nc = bacc.Bacc("TRN2", target_bir_lowering=False, debug=False, num_devices=NCORES)

    xT = nc.dram_tensor("xT", (D, TOK), F32, kind="ExternalInput").ap()
    w1 = nc.dram_tensor("w1", (L1_NK, P, H), F16, kind="ExternalInput").ap()
    w2 = nc.dram_tensor("w2", (L2_NK, P, O), F16, kind="ExternalInput").ap()
    out = nc.dram_tensor("out", (TOK, O), F32, kind="ExternalOutput").ap()

    W1FREE = 4 * TOK  # L1 mega-tile free dim (all 4 input blocks)
    W2FREE = 8 * TOK  # L2 mega-tile free dim (all 8 hidden blocks)

    with tile.TileContext(nc) as tc, ExitStack() as ctx:
        persist = ctx.enter_context(tc.tile_pool(name="persist", bufs=1))
        tmp = ctx.enter_context(tc.tile_pool(name="tmp", bufs=2))
        rp = ctx.enter_context(tc.tile_pool(name="rp", bufs=3))
        w1p = ctx.enter_context(tc.tile_pool(name="w1p", bufs=4))
        w2p = ctx.enter_context(tc.tile_pool(name="w2p", bufs=6))
        outp = ctx.enter_context(tc.tile_pool(name="outp", bufs=4))
        psum = ctx.enter_context(tc.tile_pool(name="psum", bufs=1, space="PSUM"))

        _bias_cache: dict = {}

        def bias_ap(val: float):
            if val not in _bias_cache:
                t = persist.tile([P, 1], F32, tag=f"bias{len(_bias_cache)}",
                                 name=f"bias_{len(_bias_cache)}")
                nc.vector.memset(t, val)
                _bias_cache[val] = t
            return _bias_cache[val]

        def emit_basis(u_ap, g, free, name, suffix=""):
            """F = relu(2-a)^3 - 4*relu(1-a)^3, a = |u - (g+2)|; f16 mega."""
            c = float(g + 2)
            t = tmp.tile([P, free], F16, tag=f"pc{name}{suffix}",
                         name=f"pc{name}g{g}{suffix}")
            nc.vector._custom_dve(opP, out=t, in0=u_ap, s0=c, s1=2.0)
            f = rp.tile([P, free], F16, tag=f"qf{name}{suffix}",
                        name=f"qf{name}g{g}{suffix}")
            nc.vector._custom_dve(opQ, out=f, in0=u_ap, in1=t, s0=c, s1=1.0,
                                  imm2=-4.0)
            return f

        # ---- L1 inputs: x mega-tile + activations -----------------------
        # Sliced (not mega) activations: si1[ib] only needs x-slice ib, so
        # the first matmul starts as soon as one DMA + one ACT pass finish.
        xm = persist.tile([P, W1FREE], F32, tag="xm", name="xm")
        for ib in range(4):
            nc.sync.dma_start(
                out=xm[:, ib * TOK : (ib + 1) * TOK],
                in_=xT[ib * P : (ib + 1) * P, :],
            )

        u1 = persist.tile([P, W1FREE], F32, tag="u1", name="u1")
        si1 = persist.tile([P, W1FREE], F16, tag="si1", name="si1")
        for ib in range(4):
            sl = slice(ib * TOK, (ib + 1) * TOK)
            nc.scalar.activation(si1[:, sl], xm[:, sl], AF.Silu,
                                 bias=bias_ap(0.0))
            nc.vector.tensor_scalar(u1[:, sl], xm[:, sl], 2.5, 5.5,
                                    ALU.mult, ALU.add)

        # ---- L1 matmuls: out1[o_blk, tok] accumulated over 36 K-tiles ---
        pb = [
            psum.tile([P, TOK], F32, tag=f"p{ob}", name=f"p{ob}") for ob in range(8)
        ]

        def l1_block(k, rhs_ap):
            wt = w1p.tile([P, H], F16, tag="w1k", name=f"w1k{k}")
            nc.sync.dma_start(out=wt, in_=w1[k])
            last = k == L1_NK - 1
            for ob in range(8):
                nc.tensor.matmul(
                    pb[ob],
                    wt[:, ob * P : (ob + 1) * P],
                    rhs_ap,
                    start=(k == 0),
                    stop=last,
                )

        for k in range(4):  # silu base path
            l1_block(k, si1[:, k * TOK : (k + 1) * TOK])
        for g in range(NB):
            f = emit_basis(u1, g, W1FREE, "a")
            for ib in range(4):
                l1_block(4 + g * 4 + ib, f[:, ib * TOK : (ib + 1) * TOK])

        # ---- boundary: h = out1 in PSUM [h_blk, tok]; L2 inputs ---------
        # qb aliases PSUM banks p4..p7, so L2's first matmuls only need
        # pb[4..7] fully read.  Process j=4..7 first, silu on ACT in
        # parallel with the u-affine on DVE, so the banks free after ~3.5us
        # (inside the PE HAM idle window — the PE stays at 2.4 GHz).
        u2 = persist.tile([P, W2FREE], F32, tag="u2", name="u2")
        si2 = persist.tile([P, W2FREE], F16, tag="si2", name="si2")
        J_ORDER = [4, 5, 6, 7, 0, 1, 2, 3]
        for j in J_ORDER:
            sl = slice(j * TOK, (j + 1) * TOK)
            nc.scalar.activation(si2[:, sl], pb[j], AF.Silu, bias=bias_ap(0.0))
        for j in J_ORDER:
            sl = slice(j * TOK, (j + 1) * TOK)
            nc.vector.tensor_scalar(u2[:, sl], pb[j], 2.5, 5.5,
                                    ALU.mult, ALU.add)

        # ---- L2 matmuls: out2[tok_blk, o] over 72 K-tiles ---------------
        qb = [
            psum.tile([P, O], F32, tag=f"p{tb + 4}", name=f"q{tb}")
            for tb in range(4)
        ]

        kctr = [0]

        def l2_block(widx, lhsT_ap, last=False):
            """widx: row of the packed w2 tensor (logical K-tile identity)."""
            first = kctr[0] == 0
            kctr[0] += 1
            wt = w2p.tile([P, O], F16, tag="w2k", name=f"w2k{widx}")
            nc.sync.dma_start(out=wt, in_=w2[widx])
            for tb in range(4):
                nc.tensor.matmul(
                    qb[tb],
                    lhsT_ap[:, tb * P : (tb + 1) * P],
                    wt,
                    start=first,
                    stop=last,
                )

        for j in J_ORDER:  # silu base path, j=4..7 first
            l2_block(j, si2[:, j * TOK : (j + 1) * TOK])
        # first two basis functions in half-megas so the first L2 basis
        # matmuls don't wait for the full u2; the rest full-width.
        HALF = W2FREE // 2
        for g in range(NB):
            if g < 2:
                fh0 = emit_basis(u2[:, :HALF], g, HALF, "b", "h0")
                for j in range(4):
                    l2_block(8 + g * 8 + j, fh0[:, j * TOK : (j + 1) * TOK])
                fh1 = emit_basis(u2[:, HALF:], g, HALF, "b", "h1")
                for j in range(4, 8):
                    l2_block(8 + g * 8 + j,
                             fh1[:, (j - 4) * TOK : (j - 3) * TOK])
            else:
                f = emit_basis(u2, g, W2FREE, "b")
                for j in range(8):
                    l2_block(8 + g * 8 + j, f[:, j * TOK : (j + 1) * TOK],
                             last=(g == NB - 1 and j == 7))

        # ---- store: copies split across Scalar/Vector, then DMA out -----
        for tb in range(4):
            ot = outp.tile([P, O], F32, tag="ot", name=f"ot{tb}")
            if tb % 2 == 0:
                nc.scalar.copy(ot, qb[tb])
            else:
                nc.vector.tensor_copy(ot, qb[tb])
            nc.sync.dma_start(out=out[tb * P : (tb + 1) * P, :], in_=ot)

    nc.compile()
    _BUILD_CACHE["nc"] = nc
    return nc


# ---------------------------------------------------------------- entry
def kernel(x, coef1, scale_base1, scale_sp1, coef2, scale_base2, scale_sp2,
           _want_trace=False):
    from concourse.bass_utils import run_bass_kernel_spmd

    x_flat = np.asarray(x, np.float32).reshape(NTOK, D)
    w1 = _pack_w1(np.asarray(coef1), np.asarray(scale_sp1), np.asarray(scale_base1))
    w2 = _pack_w2(np.asarray(coef2), np.asarray(scale_sp2), np.asarray(scale_base2))

    nc = _build_kernel()

    in_maps = []
    for c in range(NCORES):
        xs = x_flat[c * TOK : (c + 1) * TOK]  # (TOK, D)
        in_maps.append(
            {
                "xT": np.ascontiguousarray(xs.T),
                "w1": w1,
                "w2": w2,
            }
        )

    res = run_bass_kernel_spmd(
        nc, in_maps, core_ids=list(range(NCORES)), trace=_want_trace
    )
    outs = [res.results[c]["out"] for c in range(NCORES)]
    full = np.concatenate(outs, axis=0).reshape(x.shape[0], x.shape[1], O)
    if _want_trace:
        kernel._last_results = res  # stash for test harness profiling
    return full.astype(np.float32)
